# revision 16
# baseline (speedup 1.0000x reference)
"""Trainium2 Bass kernel for nn_AutoManagedAttention (B=2, S=2048, H=2048, NH=16).

Sharding: 8 cores; core c handles batch b=c//4 and 4 heads h in
[4*(c%4), 4*(c%4)+4).  Each core computes its heads' Q/K/V projections,
attention (scores computed transposed [t,q] so the V-matmul needs no on-chip
transposes), softmax denominators via an all-ones matmul, normalized
attn_weights written via PE-transpose, and a partial output projection.
Host sums the 4 per-batch partials and adds the output bias.

KERNEL_MM_MODE selects the matmul operand dtype: f16 (default; 1 cyc/row on
the PE and ~7e-4 end-to-end max rel err), f32r (TF32-like, ~1.5 cyc/row,
~4e-4), or f32 (4 cyc/row). attn_weights and out are always written fp32.
"""

import os

import numpy as np

B, S, H, NH, HD = 2, 2048, 2048, 16, 128
P = 128
CROSS_STEP, R2R = 0.7, 0.8
NCORES = 8
UPC = 4  # heads (units) per core
MW = UPC * HD  # 512: per-core width of the head-concat (m) axis
MO = MW // P  # 4
KO = H // P  # 16 contraction tiles for the projections
QCH = 512  # q-chunk in attention
NQC = S // QCH  # 4
NT = S // P  # 16 t-tiles
SCH = 512  # s-chunk in phase 1
NSC = S // SCH  # 4
SCALE = float(HD) ** -0.5

_CACHE = {}


def _build(mm_fast: str):
    import concourse.bacc as bacc
    import concourse.mybir as mybir
    import concourse.tile as tile
    from concourse.masks import make_identity

    f32 = mybir.dt.float32
    dtM = {"f16": mybir.dt.float16, "f32r": mybir.dt.float32r, "f32": f32}[mm_fast]
    AF = mybir.ActivationFunctionType

    nc = bacc.Bacc("TRN2", target_bir_lowering=False)

    xT = nc.dram_tensor("xT", [H, S], dtM, kind="ExternalInput")
    wqT = nc.dram_tensor("wqT", [H, MW], dtM, kind="ExternalInput")
    wkT = nc.dram_tensor("wkT", [H, MW], dtM, kind="ExternalInput")
    wvT = nc.dram_tensor("wvT", [H, MW], dtM, kind="ExternalInput")
    woT = nc.dram_tensor("woT", [MW, H], dtM, kind="ExternalInput")
    bqs = nc.dram_tensor("bqs", [MW], f32, kind="ExternalInput")
    bks = nc.dram_tensor("bks", [MW], f32, kind="ExternalInput")
    bvb = nc.dram_tensor("bvb", [P, MW], f32, kind="ExternalInput")
    btab = nc.dram_tensor("btab", [P, UPC, NT, NQC], f32, kind="ExternalInput")
    pats = nc.dram_tensor("pats", [5, P, QCH], f32, kind="ExternalInput")
    aw = nc.dram_tensor("aw", [UPC, S, S], f32, kind="ExternalOutput")
    fout = nc.dram_tensor("fout", [S, H], f32, kind="ExternalOutput")

    xT_v = xT.ap().rearrange("(ko p) s -> p ko s", p=P)
    wq_v = wqT.ap().rearrange("(ko p) m -> p ko m", p=P)
    wk_v = wkT.ap().rearrange("(ko p) m -> p ko m", p=P)
    wv_v = wvT.ap().rearrange("(ko p) m -> p ko m", p=P)
    wo_v = woT.ap().rearrange("(mo p) n -> p mo n", p=P)

    NXT = 4  # xT tiles per s-chunk

    with tile.TileContext(nc) as tc:
        with tc.tile_pool(name="persist", bufs=1) as persist:
            qt_sb = persist.tile([P, UPC, S], dtM)  # QT[d, u, s] (scaled, +bias)
            kt_sb = persist.tile([P, UPC, S], dtM)  # KT[d, u, s] (+bias)
            v_sb = persist.tile([P, NT, MW], dtM)  # V[t_in, t_out, m] (+bias)
            ones_sb = persist.tile([P, P], dtM)
            ident_sb = persist.tile([P, P], dtM)
            pat_sb = persist.tile([P, 5, QCH], f32)
            btab_sb = persist.tile([P, UPC, NT, NQC], f32)

            # ---------------- phase 1: projections ----------------
            with (
                tc.tile_pool(name="p1w", bufs=1) as p1w,
                tc.tile_pool(name="p1s", bufs=3) as p1s,
                tc.tile_pool(name="p1x", bufs=NXT + 2) as p1x,
                tc.tile_pool(name="p1ps", bufs=3, space="PSUM") as p1ps,
                tc.tile_pool(name="p1psv", bufs=4, space="PSUM") as p1psv,
            ):
                if mm_fast != "f32":
                    ones_f = p1w.tile([P, P], f32)
                    ident_f = p1w.tile([P, P], f32)
                    nc.vector.memset(ones_f[:], 1.0)
                    make_identity(nc, ident_f[:])
                    nc.vector.tensor_copy(ones_sb[:], ones_f[:])
                    nc.vector.tensor_copy(ident_sb[:], ident_f[:])
                else:
                    nc.vector.memset(ones_sb[:], 1.0)
                    make_identity(nc, ident_sb[:])
                wq_sb = p1w.tile([P, KO, MW], dtM)
                wk_sb = p1w.tile([P, KO, MW], dtM)
                wv_sb = p1w.tile([P, KO, MW], dtM)
                # split the loads so the first matmuls start early
                for kg in range(4):
                    nc.sync.dma_start(
                        wq_sb[:, kg * 4 : (kg + 1) * 4, :],
                        wq_v[:, kg * 4 : (kg + 1) * 4, :],
                    )

                bq_sb = p1w.tile([P, MO], f32)
                bk_sb = p1w.tile([P, MO], f32)
                bv_sb = p1w.tile([P, MW], f32)
                nc.sync.dma_start(bq_sb[:], bqs.ap().rearrange("(mo p) -> p mo", p=P))
                nc.sync.dma_start(bk_sb[:], bks.ap().rearrange("(mo p) -> p mo", p=P))
                nc.sync.dma_start(bv_sb[:], bvb.ap())

                for sc in range(NSC):
                    ssl = slice(sc * SCH, (sc + 1) * SCH)
                    xts = []
                    for part in range(NXT):
                        xt = p1x.tile(
                            [P, KO // NXT, SCH], dtM, tag="xt", name=f"xt{part}"
                        )
                        nc.sync.dma_start(
                            xt[:],
                            xT_v[:, part * (KO // NXT) : (part + 1) * (KO // NXT), ssl],
                        )
                        xts.append(xt)

                    def xtile(ko):
                        return xts[ko // (KO // NXT)][:, ko % (KO // NXT), :]

                    # Q: mo-outer, wq resident
                    for mo in range(MO):
                        ps = p1ps.tile([P, SCH], f32, tag="ps", name=f"ps{mo}")
                        for ko in range(KO):
                            nc.tensor.matmul(
                                ps[:],
                                wq_sb[:, ko, mo * P : (mo + 1) * P],
                                xtile(ko),
                                start=(ko == 0),
                                stop=(ko == KO - 1),
                            )
                        nc.scalar.activation(
                            qt_sb[:, mo, ssl], ps[:], AF.Identity,
                            bias=bq_sb[:, mo : mo + 1], scale=SCALE,
                        )
                    if sc == 0:
                        for w_dst, w_src in ((wk_sb, wk_v), (wv_sb, wv_v)):
                            for kg in range(4):
                                nc.sync.dma_start(
                                    w_dst[:, kg * 4 : (kg + 1) * 4, :],
                                    w_src[:, kg * 4 : (kg + 1) * 4, :],
                                )
                        nc.sync.dma_start(
                            pat_sb[:], pats.ap().rearrange("c p q -> p c q")
                        )
                        nc.sync.dma_start(btab_sb[:], btab.ap())
                    # K: ko-outer, wk resident, 4 psum banks
                    kps = [
                        p1psv.tile([P, SCH], f32, tag="kv4", name=f"kps{mo}")
                        for mo in range(MO)
                    ]
                    for ko in range(KO):
                        for mo in range(MO):
                            nc.tensor.matmul(
                                kps[mo][:],
                                wk_sb[:, ko, mo * P : (mo + 1) * P],
                                xtile(ko),
                                start=(ko == 0),
                                stop=(ko == KO - 1),
                            )
                    for mo in range(MO):
                        nc.scalar.activation(
                            kt_sb[:, mo, ssl], kps[mo][:], AF.Identity,
                            bias=bk_sb[:, mo : mo + 1], scale=1.0,
                        )
                    # V in natural [t, m] layout: ko-outer, streamed wv
                    vps = [
                        p1psv.tile([P, MW], f32, tag="kv4", name=f"vps{st}")
                        for st in range(SCH // P)
                    ]
                    for ko in range(KO):
                        for st in range(SCH // P):
                            nc.tensor.matmul(
                                vps[st][:],
                                xtile(ko)[:, st * P : (st + 1) * P],
                                wv_sb[:, ko, :],
                                start=(ko == 0),
                                stop=(ko == KO - 1),
                            )
                    for st in range(SCH // P):
                        nc.vector.tensor_add(
                            v_sb[:, sc * (SCH // P) + st, :], vps[st][:], bv_sb[:]
                        )

            # ---------------- phases 2+3: attention ----------------
            with tc.tile_pool(name="pattn", bufs=1) as pattn:
                ao_sb = pattn.tile([P, MO, S], dtM)  # normalized attn-out^T [d, u, q]
                wo_sb = pattn.tile([P, MO, H], dtM)
                for nch in range(H // 512):
                    for mo in range(MO):
                        nsl = slice(nch * 512, (nch + 1) * 512)
                        nc.sync.dma_start(wo_sb[:, mo, nsl], wo_v[:, mo, nsl])

                with (
                    tc.tile_pool(name="pexp", bufs=24) as pexp,
                    tc.tile_pool(name="pawt", bufs=3) as pawt,
                    tc.tile_pool(name="prec", bufs=2) as prec,
                    tc.tile_pool(name="psS", bufs=2, space="PSUM") as psS,
                    tc.tile_pool(name="psAo", bufs=2, space="PSUM") as psAo,
                    tc.tile_pool(name="psAs", bufs=2, space="PSUM") as psAs,
                    tc.tile_pool(name="psG", bufs=2, space="PSUM") as psG,
                ):
                    # one-chunk software pipeline: while chunk c's t-loop
                    # (scores/exp/sums/outv) runs, chunk c-1's drain
                    # (aw transposes + normalize copyback + DMA) interleaves,
                    # keeping the PE fed across chunk boundaries.
                    def a_iter(st, t):
                        u, qc = st["u"], st["qc"]
                        qsl = slice(qc * QCH, (qc + 1) * QCH)
                        sc_ps = psS.tile([P, QCH], f32, tag="sc", name="sc")
                        nc.tensor.matmul(
                            sc_ps[:],
                            kt_sb[:, u, t * P : (t + 1) * P],
                            qt_sb[:, u, qsl],
                            start=True,
                            stop=True,
                        )
                        o128 = t - (QCH // P) * qc
                        if o128 < 0:
                            nc.vector.tensor_add(sc_ps[:], sc_ps[:], pat_sb[:, 0, :])
                        elif o128 <= 3:
                            nc.vector.tensor_add(
                                sc_ps[:], sc_ps[:], pat_sb[:, 1 + o128, :]
                            )
                        e = pexp.tile([P, QCH], dtM, tag="exp", name="exp")
                        nc.scalar.activation(
                            e[:], sc_ps[:], AF.Exp,
                            bias=btab_sb[:, u, t, qc : qc + 1], scale=1.0,
                        )
                        st["exps"].append(e)
                        nc.tensor.matmul(
                            st["sums"][:], ones_sb[:], e[:],
                            start=(t == 0), stop=(t == NT - 1),
                        )
                        nc.tensor.matmul(
                            st["outv"][:],
                            v_sb[:, t, u * HD : (u + 1) * HD],
                            e[:],
                            start=(t == 0),
                            stop=(t == NT - 1),
                        )

                    def b_prolog(st):
                        u, qc = st["u"], st["qc"]
                        qsl = slice(qc * QCH, (qc + 1) * QCH)
                        sums_sb = prec.tile(
                            [P, QCH], dtM, tag="sums_sb", name="sums_sb"
                        )
                        nc.scalar.copy(sums_sb[:], st["sums"][:])
                        rec_sb = prec.tile([P, QCH], f32, tag="rec", name="rec")
                        nc.vector.reciprocal(rec_sb[:], sums_sb[:])
                        nc.vector.tensor_mul(ao_sb[:, u, qsl], st["outv"][:], rec_sb[:])
                        st["rcols"] = []
                        for qs in range(QCH // P):
                            rt_ps = psG.tile([P, 4 * P], f32, tag="tr", name="rt")
                            nc.tensor.matmul(
                                rt_ps[:, :P], sums_sb[:, qs * P : (qs + 1) * P],
                                ident_sb[:], start=True, stop=True,
                            )
                            rcol = prec.tile([P, 1], f32, tag="rcol", name="rcol", bufs=8)
                            nc.vector.reciprocal(rcol[:], rt_ps[:, 0:1])
                            st["rcols"].append(rcol)

                    def b_group(st, g):
                        u, qc = st["u"], st["qc"]
                        qs, tg = g // (NT // 4), g % (NT // 4)
                        if tg == 0:
                            st["awt"] = pawt.tile([P, S], f32, tag="awt", name="awt")
                        awt_sb = st["awt"]
                        rcol = st["rcols"][qs]
                        tr_ps = psG.tile([P, 4 * P], f32, tag="tr", name="tr")
                        for j in range(4):
                            nc.tensor.matmul(
                                tr_ps[:, j * P : (j + 1) * P],
                                st["exps"][tg * 4 + j][:, qs * P : (qs + 1) * P],
                                ident_sb[:], start=True, stop=True,
                            )
                        osl = slice(tg * 4 * P, (tg + 1) * 4 * P)
                        if tg % 2 == 0:
                            nc.scalar.activation(
                                awt_sb[:, osl], tr_ps[:], AF.Copy,
                                bias=0.0, scale=rcol[:],
                            )
                        else:
                            nc.vector.tensor_scalar_mul(
                                awt_sb[:, osl], tr_ps[:], rcol[:]
                            )
                        if tg == NT // 4 - 1:
                            q0 = qc * QCH + qs * P
                            nc.sync.dma_start(
                                aw.ap()[u, q0 : q0 + P, :], awt_sb[:]
                            )

                    # chunk c's drain is emitted LAG iterations into chunk
                    # c+1's t-loop so the in-order PE FIFO reaches the next
                    # chunk's scores before the rt/tr work that waits on ACT.
                    LAG = 3
                    prev = None
                    for u in range(UPC):
                        for qc in range(NQC):
                            st = {
                                "u": u,
                                "qc": qc,
                                "exps": [],
                                "sums": psAs.tile([P, QCH], f32, tag="sums", name="sums"),
                                "outv": psAo.tile([P, QCH], f32, tag="outv", name="outv"),
                            }
                            for t in range(NT):
                                a_iter(st, t)
                                if prev is not None:
                                    if t == LAG - 1:
                                        b_prolog(prev)
                                    elif t >= LAG:
                                        b_group(prev, t - LAG)
                            if prev is not None:
                                for g in range(NT - LAG, NT):
                                    b_group(prev, g)
                            prev = st
                    b_prolog(prev)
                    for g in range(NT):
                        b_group(prev, g)

                # ---------------- phase 4: output projection ----------------
                with (
                    tc.tile_pool(name="p4w", bufs=1) as p4w,
                    tc.tile_pool(name="p4s", bufs=3) as p4s,
                    tc.tile_pool(name="p4ps", bufs=3, space="PSUM") as p4ps,
                ):
                    for st in range(S // P):
                        for nch in range(H // 512):
                            ps = p4ps.tile([P, 512], f32, tag="fps", name="fps")
                            for mo in range(MO):
                                nc.tensor.matmul(
                                    ps[:],
                                    ao_sb[:, mo, st * P : (st + 1) * P],
                                    wo_sb[:, mo, nch * 512 : (nch + 1) * 512],
                                    start=(mo == 0),
                                    stop=(mo == MO - 1),
                                )
                            ft = p4s.tile([P, 512], f32, tag="ft", name="ft")
                            nc.scalar.copy(ft[:], ps[:])
                            nc.sync.dma_start(
                                fout.ap()[st * P : (st + 1) * P,
                                          nch * 512 : (nch + 1) * 512],
                                ft[:],
                            )

    nc.finalize()
    return nc


def _softmax(v):
    v = np.asarray(v, np.float32)
    e = np.exp(v - v.max())
    return e / e.sum()


def kernel(**inputs):
    from concourse.bass_utils import run_bass_kernel_spmd

    x = np.asarray(inputs["x"], np.float32)
    wq, bq = np.asarray(inputs["wq"]), np.asarray(inputs["bq"])
    wk, bk = np.asarray(inputs["wk"]), np.asarray(inputs["bk"])
    wv, bv = np.asarray(inputs["wv"]), np.asarray(inputs["bv"])
    wo, bo = np.asarray(inputs["wo"]), np.asarray(inputs["bo"])
    attn_bias = np.asarray(inputs["attn_bias"]).reshape(NH)
    flow_weights = np.asarray(inputs["flow_weights"])

    mm_mode = os.environ.get("KERNEL_MM_MODE", "f16")
    trace = os.environ.get("KERNEL_TRACE", "0") == "1"
    if trace:
        try:
            import axon_profile_shim  # noqa: F401
        except ImportError:
            pass

    key = mm_mode
    if key not in _CACHE:
        _CACHE[key] = _build(mm_fast=mm_mode)
    nc = _CACHE[key]

    fw = _softmax(flow_weights)
    c_mod = float(fw[2]) * 0.3 * CROSS_STEP * R2R / float(S)

    # score-modifier pattern tiles for the [t,q]-layout psum tiles
    tt = np.arange(P, dtype=np.float32)[:, None]
    qq = np.arange(QCH, dtype=np.float32)[None, :]
    pats = np.zeros((5, P, QCH), np.float32)
    pats[0] = c_mod * (qq - tt)  # full lower tiles (o<0); -c_mod*o folded in btab
    for oi in range(4):
        o = oi * P
        pats[1 + oi] = np.where(tt + o < qq, c_mod * (qq - tt - o), 0.0)

    mdt = np.float16 if mm_mode == "f16" else np.float32
    xTs = [np.ascontiguousarray(x[b].T).astype(mdt) for b in range(B)]
    wqT, wkT, wvT, woTf = wq.T, wk.T, wv.T, wo.T

    in_maps = []
    for c in range(NCORES):
        b = c // 4
        m0 = (c % 4) * MW
        msl = slice(m0, m0 + MW)
        heads = [4 * (c % 4) + u for u in range(UPC)]
        btab_c = np.zeros((P, UPC, NT, NQC), np.float32)
        for u, h in enumerate(heads):
            ab = float(attn_bias[h])
            for t in range(NT):
                for qc in range(NQC):
                    o = (t - (QCH // P) * qc) * P
                    btab_c[:, u, t, qc] = ab + (c_mod * (-o) if o < 0 else 0.0)
        in_maps.append(
            {
                "xT": xTs[b],
                "wqT": np.ascontiguousarray(wqT[:, msl]).astype(mdt),
                "wkT": np.ascontiguousarray(wkT[:, msl]).astype(mdt),
                "wvT": np.ascontiguousarray(wvT[:, msl]).astype(mdt),
                "woT": np.ascontiguousarray(woTf[msl, :]).astype(mdt),
                "bqs": np.ascontiguousarray(bq[msl] * SCALE),
                "bks": np.ascontiguousarray(bk[msl]),
                "bvb": np.broadcast_to(bv[msl], (P, MW)).copy(),
                "btab": btab_c,
                "pats": pats,
            }
        )

    res = run_bass_kernel_spmd(
        nc, in_maps, core_ids=list(range(NCORES)), trace=trace
    )
    kernel.last_exec_time_ns = res.exec_time_ns

    out = np.zeros((B, S, H), np.float32)
    awf = np.empty((B, NH, S, S), np.float32)
    for c in range(NCORES):
        b = c // 4
        h0 = 4 * (c % 4)
        awf[b, h0 : h0 + UPC] = res.results[c]["aw"]
        out[b] += res.results[c]["fout"]
    out += bo.astype(np.float32)
    return out, awf


# revision 18
# speedup vs baseline: 1.0618x; 1.0618x over previous
"""Trainium2 Bass kernel for nn_AutoManagedAttention (B=2, S=2048, H=2048, NH=16).

Sharding: 8 cores; core c handles batch b=c//4 and 4 heads h in
[4*(c%4), 4*(c%4)+4).  Each core computes its heads' Q/K/V projections,
attention (scores computed transposed [t,q] so the V-matmul needs no on-chip
transposes), softmax denominators via an all-ones matmul, normalized
attn_weights written via PE-transpose, and a partial output projection.
Host sums the 4 per-batch partials and adds the output bias.

KERNEL_MM_MODE selects the matmul operand dtype: f16 (default; 1 cyc/row on
the PE and ~7e-4 end-to-end max rel err), f32r (TF32-like, ~1.5 cyc/row,
~4e-4), or f32 (4 cyc/row). attn_weights and out are always written fp32.
"""

import os

import numpy as np

B, S, H, NH, HD = 2, 2048, 2048, 16, 128
P = 128
CROSS_STEP, R2R = 0.7, 0.8
NCORES = 8
UPC = 4  # heads (units) per core
MW = UPC * HD  # 512: per-core width of the head-concat (m) axis
MO = MW // P  # 4
KO = H // P  # 16 contraction tiles for the projections
QCH = 512  # q-chunk in attention
NQC = S // QCH  # 4
NT = S // P  # 16 t-tiles
SCH = 512  # s-chunk in phase 1
NSC = S // SCH  # 4
SCALE = float(HD) ** -0.5

_CACHE = {}


def _build(mm_fast: str):
    import concourse.bacc as bacc
    import concourse.mybir as mybir
    import concourse.tile as tile
    from concourse.masks import make_identity

    f32 = mybir.dt.float32
    dtM = {"f16": mybir.dt.float16, "f32r": mybir.dt.float32r, "f32": f32}[mm_fast]
    AF = mybir.ActivationFunctionType

    nc = bacc.Bacc("TRN2", target_bir_lowering=False)

    xT = nc.dram_tensor("xT", [H, S], dtM, kind="ExternalInput")
    wqT = nc.dram_tensor("wqT", [H, MW], dtM, kind="ExternalInput")
    wkT = nc.dram_tensor("wkT", [H, MW], dtM, kind="ExternalInput")
    wvT = nc.dram_tensor("wvT", [H, MW], dtM, kind="ExternalInput")
    woT = nc.dram_tensor("woT", [MW, H], dtM, kind="ExternalInput")
    bqs = nc.dram_tensor("bqs", [MW], f32, kind="ExternalInput")
    bks = nc.dram_tensor("bks", [MW], f32, kind="ExternalInput")
    bvb = nc.dram_tensor("bvb", [P, MW], f32, kind="ExternalInput")
    btab = nc.dram_tensor("btab", [P, UPC, NT, NQC], f32, kind="ExternalInput")
    pats = nc.dram_tensor("pats", [5, P, QCH], f32, kind="ExternalInput")
    aw = nc.dram_tensor("aw", [UPC, S, S], f32, kind="ExternalOutput")
    fout = nc.dram_tensor("fout", [S, H], f32, kind="ExternalOutput")

    xT_v = xT.ap().rearrange("(ko p) s -> p ko s", p=P)
    wq_v = wqT.ap().rearrange("(ko p) m -> p ko m", p=P)
    wk_v = wkT.ap().rearrange("(ko p) m -> p ko m", p=P)
    wv_v = wvT.ap().rearrange("(ko p) m -> p ko m", p=P)
    wo_v = woT.ap().rearrange("(mo p) n -> p mo n", p=P)

    NXT = 4  # xT tiles per s-chunk

    with tile.TileContext(nc) as tc:
        with tc.tile_pool(name="persist", bufs=1) as persist:
            qt_sb = persist.tile([P, UPC, S], dtM)  # QT[d, u, s] (scaled, +bias)
            kt_sb = persist.tile([P, UPC, S], dtM)  # KT[d, u, s] (+bias)
            v_sb = persist.tile([P, NT, MW], dtM)  # V[t_in, t_out, m] (+bias)
            ones_sb = persist.tile([P, P], dtM)
            ident_sb = persist.tile([P, P], dtM)
            pat_sb = persist.tile([P, 5, QCH], f32)
            btab_sb = persist.tile([P, UPC, NT, NQC], f32)

            # ---------------- phase 1: projections ----------------
            with (
                tc.tile_pool(name="p1w", bufs=1) as p1w,
                tc.tile_pool(name="p1s", bufs=3) as p1s,
                tc.tile_pool(name="p1x", bufs=NXT + 2) as p1x,
                tc.tile_pool(name="p1ps", bufs=3, space="PSUM") as p1ps,
                tc.tile_pool(name="p1psv", bufs=4, space="PSUM") as p1psv,
            ):
                if mm_fast != "f32":
                    ones_f = p1w.tile([P, P], f32)
                    ident_f = p1w.tile([P, P], f32)
                    nc.vector.memset(ones_f[:], 1.0)
                    make_identity(nc, ident_f[:])
                    nc.vector.tensor_copy(ones_sb[:], ones_f[:])
                    nc.vector.tensor_copy(ident_sb[:], ident_f[:])
                else:
                    nc.vector.memset(ones_sb[:], 1.0)
                    make_identity(nc, ident_sb[:])
                wq_sb = p1w.tile([P, KO, MW], dtM)
                wk_sb = p1w.tile([P, KO, MW], dtM)
                wv_sb = p1w.tile([P, KO, MW], dtM)
                # split the loads so the first matmuls start early
                for kg in range(4):
                    nc.sync.dma_start(
                        wq_sb[:, kg * 4 : (kg + 1) * 4, :],
                        wq_v[:, kg * 4 : (kg + 1) * 4, :],
                    )

                bq_sb = p1w.tile([P, MO], f32)
                bk_sb = p1w.tile([P, MO], f32)
                bv_sb = p1w.tile([P, MW], f32)
                nc.sync.dma_start(bq_sb[:], bqs.ap().rearrange("(mo p) -> p mo", p=P))
                nc.sync.dma_start(bk_sb[:], bks.ap().rearrange("(mo p) -> p mo", p=P))
                nc.sync.dma_start(bv_sb[:], bvb.ap())

                for sc in range(NSC):
                    ssl = slice(sc * SCH, (sc + 1) * SCH)
                    xts = []
                    for part in range(NXT):
                        xt = p1x.tile(
                            [P, KO // NXT, SCH], dtM, tag="xt", name=f"xt{part}"
                        )
                        nc.sync.dma_start(
                            xt[:],
                            xT_v[:, part * (KO // NXT) : (part + 1) * (KO // NXT), ssl],
                        )
                        xts.append(xt)

                    def xtile(ko):
                        return xts[ko // (KO // NXT)][:, ko % (KO // NXT), :]

                    # Q: mo-outer, wq resident
                    for mo in range(MO):
                        ps = p1ps.tile([P, SCH], f32, tag="ps", name=f"ps{mo}")
                        for ko in range(KO):
                            nc.tensor.matmul(
                                ps[:],
                                wq_sb[:, ko, mo * P : (mo + 1) * P],
                                xtile(ko),
                                start=(ko == 0),
                                stop=(ko == KO - 1),
                            )
                        nc.scalar.activation(
                            qt_sb[:, mo, ssl], ps[:], AF.Identity,
                            bias=bq_sb[:, mo : mo + 1], scale=SCALE,
                        )
                    if sc == 0:
                        for w_dst, w_src in ((wk_sb, wk_v), (wv_sb, wv_v)):
                            for kg in range(4):
                                nc.sync.dma_start(
                                    w_dst[:, kg * 4 : (kg + 1) * 4, :],
                                    w_src[:, kg * 4 : (kg + 1) * 4, :],
                                )
                        nc.sync.dma_start(
                            pat_sb[:], pats.ap().rearrange("c p q -> p c q")
                        )
                        nc.sync.dma_start(btab_sb[:], btab.ap())
                    # K: ko-outer, wk resident, 4 psum banks
                    kps = [
                        p1psv.tile([P, SCH], f32, tag="kv4", name=f"kps{mo}")
                        for mo in range(MO)
                    ]
                    for ko in range(KO):
                        for mo in range(MO):
                            nc.tensor.matmul(
                                kps[mo][:],
                                wk_sb[:, ko, mo * P : (mo + 1) * P],
                                xtile(ko),
                                start=(ko == 0),
                                stop=(ko == KO - 1),
                            )
                    for mo in range(MO):
                        nc.scalar.activation(
                            kt_sb[:, mo, ssl], kps[mo][:], AF.Identity,
                            bias=bk_sb[:, mo : mo + 1], scale=1.0,
                        )
                    # V in natural [t, m] layout: ko-outer, streamed wv
                    vps = [
                        p1psv.tile([P, MW], f32, tag="kv4", name=f"vps{st}")
                        for st in range(SCH // P)
                    ]
                    for ko in range(KO):
                        for st in range(SCH // P):
                            nc.tensor.matmul(
                                vps[st][:],
                                xtile(ko)[:, st * P : (st + 1) * P],
                                wv_sb[:, ko, :],
                                start=(ko == 0),
                                stop=(ko == KO - 1),
                            )
                    for st in range(SCH // P):
                        nc.vector.tensor_add(
                            v_sb[:, sc * (SCH // P) + st, :], vps[st][:], bv_sb[:]
                        )

            # ---------------- phases 2+3: attention ----------------
            with tc.tile_pool(name="pattn", bufs=1) as pattn:
                ao_sb = pattn.tile([P, MO, S], dtM)  # normalized attn-out^T [d, u, q]
                wo_sb = pattn.tile([P, MO, H], dtM)
                for nch in range(H // 512):
                    for mo in range(MO):
                        nsl = slice(nch * 512, (nch + 1) * 512)
                        nc.sync.dma_start(wo_sb[:, mo, nsl], wo_v[:, mo, nsl])

                with (
                    tc.tile_pool(name="pexp", bufs=32) as pexp,
                    tc.tile_pool(name="pawt", bufs=4) as pawt,
                    tc.tile_pool(name="prec", bufs=2) as prec,
                    tc.tile_pool(name="psS", bufs=3, space="PSUM") as psS,
                    tc.tile_pool(name="psAo", bufs=2, space="PSUM") as psAo,
                    tc.tile_pool(name="psAs", bufs=1, space="PSUM") as psAs,
                    tc.tile_pool(name="psG", bufs=2, space="PSUM") as psG,
                ):
                    # one-chunk software pipeline: while chunk c's t-loop
                    # (scores/exp/sums/outv) runs, chunk c-1's drain
                    # (aw transposes + normalize copyback + DMA) interleaves,
                    # keeping the PE fed across chunk boundaries.
                    def a_iter(st, t):
                        u, qc = st["u"], st["qc"]
                        qsl = slice(qc * QCH, (qc + 1) * QCH)
                        sc_ps = psS.tile([P, QCH], f32, tag="sc", name="sc")
                        nc.tensor.matmul(
                            sc_ps[:],
                            kt_sb[:, u, t * P : (t + 1) * P],
                            qt_sb[:, u, qsl],
                            start=True,
                            stop=True,
                        )
                        o128 = t - (QCH // P) * qc
                        if o128 < 0:
                            nc.vector.tensor_add(sc_ps[:], sc_ps[:], pat_sb[:, 0, :])
                        elif o128 <= 3:
                            nc.vector.tensor_add(
                                sc_ps[:], sc_ps[:], pat_sb[:, 1 + o128, :]
                            )
                        e = pexp.tile([P, QCH], dtM, tag="exp", name="exp")
                        nc.scalar.activation(
                            e[:], sc_ps[:], AF.Exp,
                            bias=btab_sb[:, u, t, qc : qc + 1], scale=1.0,
                        )
                        st["exps"].append(e)
                        nc.tensor.matmul(
                            st["sums"][:], ones_sb[:], e[:],
                            start=(t == 0), stop=(t == NT - 1),
                        )
                        nc.tensor.matmul(
                            st["outv"][:],
                            v_sb[:, t, u * HD : (u + 1) * HD],
                            e[:],
                            start=(t == 0),
                            stop=(t == NT - 1),
                        )

                    def b_prolog(st):
                        u, qc = st["u"], st["qc"]
                        qsl = slice(qc * QCH, (qc + 1) * QCH)
                        sums_sb = prec.tile(
                            [P, QCH], dtM, tag="sums_sb", name="sums_sb"
                        )
                        nc.vector.tensor_copy(sums_sb[:], st["sums"][:])
                        rec_sb = prec.tile([P, QCH], f32, tag="rec", name="rec")
                        nc.vector.reciprocal(rec_sb[:], sums_sb[:])
                        nc.vector.tensor_mul(ao_sb[:, u, qsl], st["outv"][:], rec_sb[:])
                        st["rcols"] = []
                        for qs in range(QCH // P):
                            rt_ps = psG.tile([P, 4 * P], f32, tag="tr", name="rt")
                            nc.tensor.matmul(
                                rt_ps[:, :P], sums_sb[:, qs * P : (qs + 1) * P],
                                ident_sb[:], start=True, stop=True,
                            )
                            rcol = prec.tile([P, 1], f32, tag="rcol", name="rcol", bufs=8)
                            nc.vector.reciprocal(rcol[:], rt_ps[:, 0:1])
                            st["rcols"].append(rcol)

                    def b_group(st, g):
                        u, qc = st["u"], st["qc"]
                        qs, tg = g // (NT // 4), g % (NT // 4)
                        if tg == 0:
                            st["awt"] = pawt.tile([P, S], f32, tag="awt", name="awt")
                        awt_sb = st["awt"]
                        rcol = st["rcols"][qs]
                        tr_ps = psG.tile([P, 4 * P], f32, tag="tr", name="tr")
                        for j in range(4):
                            nc.tensor.matmul(
                                tr_ps[:, j * P : (j + 1) * P],
                                st["exps"][tg * 4 + j][:, qs * P : (qs + 1) * P],
                                ident_sb[:], start=True, stop=True,
                            )
                        osl = slice(tg * 4 * P, (tg + 1) * 4 * P)
                        if tg % 2 == 0:
                            nc.scalar.activation(
                                awt_sb[:, osl], tr_ps[:], AF.Copy,
                                bias=0.0, scale=rcol[:],
                            )
                        else:
                            nc.vector.tensor_scalar_mul(
                                awt_sb[:, osl], tr_ps[:], rcol[:]
                            )
                        if tg == NT // 4 - 1:
                            q0 = qc * QCH + qs * P
                            nc.sync.dma_start(
                                aw.ap()[u, q0 : q0 + P, :], awt_sb[:]
                            )

                    prev = None
                    for u in range(UPC):
                        for qc in range(NQC):
                            st = {
                                "u": u,
                                "qc": qc,
                                "exps": [],
                                "sums": psAs.tile([P, QCH], f32, tag="sums", name="sums"),
                                "outv": psAo.tile([P, QCH], f32, tag="outv", name="outv"),
                            }
                            for t in range(NT):
                                a_iter(st, t)
                                if prev is not None:
                                    b_group(prev, t)
                            b_prolog(st)
                            prev = st
                    for g in range(NT):
                        b_group(prev, g)

                # ---------------- phase 4: output projection ----------------
                with (
                    tc.tile_pool(name="p4w", bufs=1) as p4w,
                    tc.tile_pool(name="p4s", bufs=3) as p4s,
                    tc.tile_pool(name="p4ps", bufs=3, space="PSUM") as p4ps,
                ):
                    for st in range(S // P):
                        for nch in range(H // 512):
                            ps = p4ps.tile([P, 512], f32, tag="fps", name="fps")
                            for mo in range(MO):
                                nc.tensor.matmul(
                                    ps[:],
                                    ao_sb[:, mo, st * P : (st + 1) * P],
                                    wo_sb[:, mo, nch * 512 : (nch + 1) * 512],
                                    start=(mo == 0),
                                    stop=(mo == MO - 1),
                                )
                            ft = p4s.tile([P, 512], f32, tag="ft", name="ft")
                            nc.scalar.copy(ft[:], ps[:])
                            nc.sync.dma_start(
                                fout.ap()[st * P : (st + 1) * P,
                                          nch * 512 : (nch + 1) * 512],
                                ft[:],
                            )

    nc.finalize()
    return nc


def _softmax(v):
    v = np.asarray(v, np.float32)
    e = np.exp(v - v.max())
    return e / e.sum()


def kernel(**inputs):
    from concourse.bass_utils import run_bass_kernel_spmd

    x = np.asarray(inputs["x"], np.float32)
    wq, bq = np.asarray(inputs["wq"]), np.asarray(inputs["bq"])
    wk, bk = np.asarray(inputs["wk"]), np.asarray(inputs["bk"])
    wv, bv = np.asarray(inputs["wv"]), np.asarray(inputs["bv"])
    wo, bo = np.asarray(inputs["wo"]), np.asarray(inputs["bo"])
    attn_bias = np.asarray(inputs["attn_bias"]).reshape(NH)
    flow_weights = np.asarray(inputs["flow_weights"])

    mm_mode = os.environ.get("KERNEL_MM_MODE", "f16")
    trace = os.environ.get("KERNEL_TRACE", "0") == "1"
    if trace:
        try:
            import axon_profile_shim  # noqa: F401
        except ImportError:
            pass

    key = mm_mode
    if key not in _CACHE:
        _CACHE[key] = _build(mm_fast=mm_mode)
    nc = _CACHE[key]

    fw = _softmax(flow_weights)
    c_mod = float(fw[2]) * 0.3 * CROSS_STEP * R2R / float(S)

    # score-modifier pattern tiles for the [t,q]-layout psum tiles
    tt = np.arange(P, dtype=np.float32)[:, None]
    qq = np.arange(QCH, dtype=np.float32)[None, :]
    pats = np.zeros((5, P, QCH), np.float32)
    pats[0] = c_mod * (qq - tt)  # full lower tiles (o<0); -c_mod*o folded in btab
    for oi in range(4):
        o = oi * P
        pats[1 + oi] = np.where(tt + o < qq, c_mod * (qq - tt - o), 0.0)

    mdt = np.float16 if mm_mode == "f16" else np.float32
    xTs = [np.ascontiguousarray(x[b].T).astype(mdt) for b in range(B)]
    wqT, wkT, wvT, woTf = wq.T, wk.T, wv.T, wo.T

    in_maps = []
    for c in range(NCORES):
        b = c // 4
        m0 = (c % 4) * MW
        msl = slice(m0, m0 + MW)
        heads = [4 * (c % 4) + u for u in range(UPC)]
        btab_c = np.zeros((P, UPC, NT, NQC), np.float32)
        for u, h in enumerate(heads):
            ab = float(attn_bias[h])
            for t in range(NT):
                for qc in range(NQC):
                    o = (t - (QCH // P) * qc) * P
                    btab_c[:, u, t, qc] = ab + (c_mod * (-o) if o < 0 else 0.0)
        in_maps.append(
            {
                "xT": xTs[b],
                "wqT": np.ascontiguousarray(wqT[:, msl]).astype(mdt),
                "wkT": np.ascontiguousarray(wkT[:, msl]).astype(mdt),
                "wvT": np.ascontiguousarray(wvT[:, msl]).astype(mdt),
                "woT": np.ascontiguousarray(woTf[msl, :]).astype(mdt),
                "bqs": np.ascontiguousarray(bq[msl] * SCALE),
                "bks": np.ascontiguousarray(bk[msl]),
                "bvb": np.broadcast_to(bv[msl], (P, MW)).copy(),
                "btab": btab_c,
                "pats": pats,
            }
        )

    res = run_bass_kernel_spmd(
        nc, in_maps, core_ids=list(range(NCORES)), trace=trace
    )
    kernel.last_exec_time_ns = res.exec_time_ns

    out = np.zeros((B, S, H), np.float32)
    awf = np.empty((B, NH, S, S), np.float32)
    for c in range(NCORES):
        b = c // 4
        h0 = 4 * (c % 4)
        awf[b, h0 : h0 + UPC] = res.results[c]["aw"]
        out[b] += res.results[c]["fout"]
    out += bo.astype(np.float32)
    return out, awf
